# revision 24
# baseline (speedup 1.0000x reference)
"""BinaryConvBNReLU Trainium2 kernel (8 NeuronCores, data-parallel over batch).

Reference computation (per nn.Module):
  bx = sign(x);  wc = clip(w, -1, 1);  alpha = mean(|wc|);  bw = sign(wc) * alpha
  out = conv2d(bx, bw, stride 1, pad 1) + x          (identity shortcut)
  out = batchnorm(out, batch stats over (B, H, W), gamma, beta, eps=1e-5)
  y = relu(out)

Strategy:
  - Batch sharded 4 images/core; weights replicated.
  - conv(sign x, sign w) on TensorE as 9 shifted bf16 matmuls per 128-channel
    chunk pair (+-1 values are exact in bf16; PSUM accumulates exact integers);
    alpha folded in at PSUM eviction: out = alpha*psum + x.
  - BN batch stats: per-core per-channel sum / sum-of-squares accumulated via
    fused accum outputs, AllReduced across the 8 cores (2 x 1KB), then
    normalize+ReLU applied on ScalarE as relu(scale*out + bias).
"""

import numpy as np

B, C, H, W = 32, 256, 56, 56
K = 3
EPS = 1e-5
N_CORES = 8
B_LOC = B // N_CORES          # 4 images per core
P = 128                       # SBUF partitions
NCH = C // P                  # 2 channel chunks
HW = H * W                    # 3136
HP, WP = H + 2, W + 2         # 58x58 zero-padded sign(x) layout
ROWS = 8                      # output rows per PSUM tile
NRT = H // ROWS               # 7 row tiles per image
NT = ROWS * W                 # 448 pixels per PSUM tile (<=512 fp32 bank)
COUNT = B * HW                # BN reduction count (global batch)

_CACHE = {}


def _build_nc():
    import concourse.bacc as bacc
    import concourse.bass_isa as bass_isa
    import concourse.mybir as mybir
    import concourse.tile as tile
    from concourse.masks import make_identity
    from contextlib import ExitStack

    f32 = mybir.dt.float32
    bf16 = mybir.dt.bfloat16
    f8 = mybir.dt.float8e4
    Alu = mybir.AluOpType
    Act = mybir.ActivationFunctionType
    AxisX = mybir.AxisListType.X
    DR = mybir.MatmulPerfMode.DoubleRow

    # flat padded sign(x) layout: BASE leading zeros + 58*58 image (+ tail pad)
    # so every (kh, kw) tap window is one contiguous run (row-wrap garbage only
    # pollutes the 2 padding columns, which eviction skips). XLEN % 16 == 0
    # keeps the fp8 DoubleRow pair-step constraint satisfied.
    BASE = 16
    XLEN = 3392  # 16 + 58*58 + 12

    nc = bacc.Bacc(
        "TRN2", target_bir_lowering=False, debug=False, num_devices=N_CORES
    )
    x_d = nc.dram_tensor("x", [B_LOC, C, H, W], f32, kind="ExternalInput")
    w_d = nc.dram_tensor("w", [C, C, K, K], f32, kind="ExternalInput")
    g_d = nc.dram_tensor("gamma", [C], f32, kind="ExternalInput")
    be_d = nc.dram_tensor("beta", [C], f32, kind="ExternalInput")
    y_d = nc.dram_tensor("y", [B_LOC, C, H, W], f32, kind="ExternalOutput")

    with tile.TileContext(nc) as tc, ExitStack() as es:
        big = es.enter_context(tc.tile_pool(name="big", bufs=1))
        wpool = es.enter_context(tc.tile_pool(name="wpool", bufs=1))
        wst = es.enter_context(tc.tile_pool(name="wst", bufs=1))
        sgt = es.enter_context(tc.tile_pool(name="sgt", bufs=2))
        xpadp = es.enter_context(tc.tile_pool(name="xpadp", bufs=B_LOC))
        psum = es.enter_context(tc.tile_pool(name="psum", bufs=4, space="PSUM"))
        psum_sq = es.enter_context(tc.tile_pool(name="psum_sq", bufs=2, space="PSUM"))
        psum_t = es.enter_context(tc.tile_pool(name="psum_t", bufs=2, space="PSUM"))
        dram = es.enter_context(tc.tile_pool(name="dram", bufs=1, space="DRAM"))

        # Entire per-core activation tensor (x, then conv+x, then relu output)
        # stays resident in SBUF: [128, 4 img, 2 chunks, 3136 px] fp32.
        out_sb = big.tile([P, B_LOC, NCH, HW], f32, name="out_sb")
        # Transposed sign weights for fp8 DoubleRow: [ci_local, tap, ci_chunk, co].
        wT8 = wpool.tile([P, K * K, NCH, C], f8, name="wT8")
        identity = wpool.tile([P, P], bf16, name="identity")
        make_identity(nc, identity)

        gamma_sb = wpool.tile([P, NCH], f32, name="gamma_sb")
        nc.sync.dma_start(gamma_sb[:], g_d.ap().rearrange("(j p) -> p j", p=P))
        beta_sb = wpool.tile([P, NCH], f32, name="beta_sb")
        nc.sync.dma_start(beta_sb[:], be_d.ap().rearrange("(j p) -> p j", p=P))

        sum_stat = wpool.tile([P, NCH, B_LOC * NRT], f32, name="sum_stat")
        sq_stat = wpool.tile([P, NCH, B_LOC * NRT], f32, name="sq_stat")
        eps_sb = wpool.tile([P, 1], f32, name="eps_sb")
        nc.vector.memset(eps_sb[:], EPS)

        # ---- weight preprocessing (chunked, interleaved with x setup) --
        w_flat = w_d.ap().rearrange("o i kh kw -> o (i kh kw)")
        a_parts = wpool.tile([P, NCH], f32, name="a_parts")
        x_flat = x_d.ap().rearrange("b c h w -> b c (h w)")
        y_flat = y_d.ap().rearrange("b c h w -> b c (h w)")
        stats_loc = wpool.tile([P, NCH, 2], f32, name="stats_loc")

        w_sbs = []
        for j in range(NCH):
            w_sb = wst.tile([P, C * K * K], f32, tag="wsb", name=f"wsb{j}")
            w_sbs.append(w_sb)

        def w_dma(j):
            nc.sync.dma_start(w_sbs[j][:], w_flat[j * P : (j + 1) * P, :])

        def w_prep(j):
            w_sb = w_sbs[j]
            w_taps = w_sb.rearrange("p (c t) -> p t c", t=K * K)
            # sign(w) -> bf16, tap-major layout [co_local, tap, ci]; one ACT op
            # per tap so TensorE transposes can start after the first tap.
            sgn = sgt.tile([P, K * K, C], bf16, tag="sgn", name=f"sgn{j}")
            for t in range(K * K):
                nc.scalar.activation(sgn[:, t, :], w_taps[:, t, :], Act.Sign)
                # transpose each [co,ci] 128x128 block on TensorE -> [ci, co]
                for k in range(NCH):
                    pt = psum_t.tile([P, P], bf16, tag="pt", name=f"pt{j}_{t}_{k}")
                    nc.tensor.transpose(pt[:], sgn[:, t, k * P : (k + 1) * P], identity[:])
                    # PSUM->SBUF copy casts to fp8; on DVE so ScalarE stays
                    # free for the x sign passes
                    nc.vector.tensor_copy(wT8[:, t, k, j * P : (j + 1) * P], pt[:])

        def w_clip_reduce(j):
            w_sb = w_sbs[j]
            # clip(w, -1, 1) in place (sign unchanged; needed for alpha only)
            nc.vector.tensor_scalar(w_sb[:], w_sb[:], 1.0, -1.0, Alu.min, Alu.max)
            nc.vector.tensor_reduce(
                a_parts[:, j : j + 1],
                w_sb[:],
                axis=AxisX,
                op=Alu.add,
                apply_absolute_value=True,
            )

        xpads = [
            xpadp.tile([P, NCH, XLEN], f8, tag="xpad", name=f"xpad{b}")
            for b in range(B_LOC)
        ]

        def x_load(b, half=None):
            # half=0/1 loads the first/second 28 rows (kickstart pipelining)
            lo = 0 if half in (None, 0) else HW // 2
            hi = HW if half in (None, 1) else HW // 2
            for k in range(NCH):
                nc.sync.dma_start(
                    out_sb[:, b, k, lo:hi], x_flat[b, k * P : (k + 1) * P, lo:hi]
                )

        def x_sign(b, half=None):
            xpad = xpads[b]
            h0 = 0 if half in (None, 0) else H // 2
            h1 = H if half in (None, 1) else H // 2
            for k in range(NCH):
                pad_img = xpad[:, k, BASE : BASE + HP * WP].rearrange(
                    "p (r c) -> p r c", c=WP
                )
                nc.scalar.activation(
                    pad_img[:, h0 + 1 : h1 + 1, 1 : W + 1],
                    out_sb[:, b, k, h0 * W : h1 * W].rearrange(
                        "p (h w) -> p h w", w=W
                    ),
                    Act.Sign,
                )

        # alpha on DVE+gpsimd only (ACT stays free for sign passes)
        a_sum = wpool.tile([P, 1], f32, name="a_sum")
        a_all = wpool.tile([P, 1], f32, name="a_all")
        alpha = wpool.tile([P, 1], f32, name="alpha")

        def alpha_finalize():
            nc.vector.tensor_reduce(a_sum[:], a_parts[:], axis=AxisX, op=Alu.add)
            nc.gpsimd.partition_all_reduce(
                a_all[:], a_sum[:], channels=P, reduce_op=bass_isa.ReduceOp.add
            )
            nc.vector.tensor_scalar_mul(alpha[:], a_all[:], 1.0 / (C * C * K * K))

        def square_unit(j, b):
            # sum-of-squares pass (ScalarE) for one (chunk, image); emission
            # deferred for early units to keep ScalarE free for sign passes
            for rt in range(NRT):
                idx = b * NRT + rt
                sq = psum_sq.tile([P, NT], f32, tag="sq", name=f"sq{b}_{j}_{rt}")
                nc.scalar.activation(
                    sq[:], out_sb[:, b, j, rt * NT : (rt + 1) * NT], Act.Square,
                    accum_out=sq_stat[:, j, idx : idx + 1],
                )

        def conv_chunk_image(j, b, squares=True):
            xpad = xpads[b]
            for rt in range(NRT):
                # padded-width output tile [8 rows, 58 cols]; cols 0 and 57 are
                # row-wrap garbage and are skipped at eviction.
                ps = psum.tile([P, ROWS, WP], f32, tag="ps", name=f"ps{b}_{j}_{rt}")
                mm = 0
                for kh in range(K):
                    for kw in range(K):
                        s = BASE + (rt * ROWS + kh) * WP + (kw - 1)
                        nc.tensor.matmul(
                            ps[:],
                            wT8[:, kh * K + kw, :, j * P : (j + 1) * P],
                            xpad[:, :, s : s + ROWS * WP],
                            start=(mm == 0),
                            stop=(mm == K * K - 1),
                            perf_mode=DR,
                        )
                        mm += 1
                idx = b * NRT + rt
                sl = out_sb[:, b, j, rt * NT : (rt + 1) * NT].rearrange(
                    "p (r c) -> p r c", c=W
                )
                # out = alpha*conv + x (in place over x), accum -> per-tile sum
                nc.vector.scalar_tensor_tensor(
                    out=sl,
                    in0=ps[:, :, 1 : W + 1],
                    scalar=alpha[:],
                    in1=sl,
                    op0=Alu.mult,
                    op1=Alu.add,
                    accum_out=sum_stat[:, j, idx : idx + 1],
                )
            if squares:
                square_unit(j, b)

        def launch_allreduce(j):
            nc.vector.tensor_reduce(
                stats_loc[:, j, 0:1], sum_stat[:, j, :], axis=AxisX, op=Alu.add
            )
            nc.vector.tensor_reduce(
                stats_loc[:, j, 1:2], sq_stat[:, j, :], axis=AxisX, op=Alu.add
            )
            bnc_in = dram.tile([P, 2], f32, name=f"bncin{j}")
            bnc_out = dram.tile([P, 2], f32, name=f"bncout{j}", addr_space="Shared")
            nc.gpsimd.dma_start(bnc_in[:], stats_loc[:, j, :])
            nc.gpsimd.collective_compute(
                "AllReduce",
                Alu.add,
                replica_groups=[list(range(N_CORES))],
                ins=[bnc_in.opt()],
                outs=[bnc_out.opt()],
            )
            glob = wpool.tile([P, 2], f32, name=f"glob{j}")
            nc.gpsimd.dma_start(glob[:], bnc_out[:])
            return glob

        def normalize_store(j, glob):
            mean = wpool.tile([P, 1], f32, name=f"mean{j}")
            nc.scalar.mul(mean[:], glob[:, 0:1], 1.0 / COUNT)
            ex2 = wpool.tile([P, 1], f32, name=f"ex2{j}")
            nc.scalar.mul(ex2[:], glob[:, 1:2], 1.0 / COUNT)
            msq = wpool.tile([P, 1], f32, name=f"msq{j}")
            nc.vector.tensor_mul(msq[:], mean[:], mean[:])
            var = wpool.tile([P, 1], f32, name=f"var{j}")
            nc.vector.tensor_sub(var[:], ex2[:], msq[:])
            sd = wpool.tile([P, 1], f32, name=f"sd{j}")
            nc.scalar.activation(sd[:], var[:], Act.Sqrt, bias=eps_sb[:])
            rinv = wpool.tile([P, 1], f32, name=f"rinv{j}")
            nc.vector.reciprocal(rinv[:], sd[:])
            scl = wpool.tile([P, 1], f32, name=f"scl{j}")
            nc.vector.tensor_mul(scl[:], rinv[:], gamma_sb[:, j : j + 1])
            mscl = wpool.tile([P, 1], f32, name=f"mscl{j}")
            nc.vector.tensor_mul(mscl[:], mean[:], scl[:])
            bia = wpool.tile([P, 1], f32, name=f"bia{j}")
            nc.vector.tensor_sub(bia[:], beta_sb[:, j : j + 1], mscl[:])
            hh = HW // 2
            for b in range(B_LOC):
                for h in range(2):
                    sl = out_sb[:, b, j, h * hh : (h + 1) * hh]
                    if (2 * b + h) % 8 < 5:
                        # ScalarE: relu(scale*x + bias) in one op
                        nc.scalar.activation(
                            sl, sl, Act.Relu, bias=bia[:], scale=scl[:]
                        )
                    else:
                        # VectorE picks up the rest in parallel (2 ops)
                        nc.vector.tensor_scalar(
                            sl, sl, scl[:], bia[:], Alu.mult, Alu.add
                        )
                        nc.vector.tensor_scalar_max(sl, sl, 0.0)
                    nc.sync.dma_start(
                        y_flat[b, j * P : (j + 1) * P, h * hh : (h + 1) * hh], sl
                    )

        # Emission order tuned so no engine FIFO blocks another engine's
        # upcoming work (every engine is strict in-order):
        #  - DMA queue: w0, x(b0), w1, x(b1..b3)
        #  - ScalarE:  w-taps j0, signs b0, w-taps j1, signs b1..b3, squares...
        #  - TensorE:  transposes j0, conv(0,b0), transposes j1, conv...
        #  - VectorE:  w-copies j0, clips/alpha, evictions, w-copies j1, ...
        # Chunk 0 finishes six units in, so its AllReduce latency (incl. peer
        # launch skew) hides under the remaining chunk-1 conv, and chunk 0's
        # normalize+store hide under the chunk-1 AllReduce wait.
        for b in range(B_LOC):
            nc.gpsimd.memset(xpads[b][:], 0.0)
        w_dma(0)
        x_load(0, half=0)
        x_load(0, half=1)
        w_dma(1)
        for b in range(1, B_LOC):
            x_load(b, half=0)
            x_load(b, half=1)
        w_prep(0)
        x_sign(0, half=0)
        x_sign(0, half=1)
        w_clip_reduce(0)
        w_clip_reduce(1)
        alpha_finalize()
        conv_chunk_image(0, 0, squares=False)
        w_prep(1)
        x_sign(1, half=0)
        x_sign(1, half=1)
        conv_chunk_image(1, 0, squares=False)
        x_sign(2, half=0)
        x_sign(2, half=1)
        conv_chunk_image(0, 1, squares=False)
        x_sign(3, half=0)
        x_sign(3, half=1)
        square_unit(0, 0)
        square_unit(1, 0)
        square_unit(0, 1)
        conv_chunk_image(1, 1)
        conv_chunk_image(0, 2)
        conv_chunk_image(1, 2)
        conv_chunk_image(0, 3)
        glob0 = launch_allreduce(0)
        conv_chunk_image(1, 3)
        glob1 = launch_allreduce(1)
        normalize_store(0, glob0)
        normalize_store(1, glob1)

    nc.compile()
    return nc


def _get_nc():
    if "nc" not in _CACHE:
        _CACHE["nc"] = _build_nc()
    return _CACHE["nc"]


def _run(in_maps, trace=False, tmpdir=None):
    import concourse.bass_utils as bass_utils

    nc = _get_nc()
    return bass_utils.run_bass_kernel_spmd(
        nc, in_maps, core_ids=list(range(N_CORES)), trace=trace, tmpdir=tmpdir
    )


def _make_in_maps(x, w, gamma, beta):
    x = np.ascontiguousarray(np.asarray(x), dtype=np.float32)
    w = np.ascontiguousarray(np.asarray(w), dtype=np.float32)
    gamma = np.ascontiguousarray(np.asarray(gamma), dtype=np.float32)
    beta = np.ascontiguousarray(np.asarray(beta), dtype=np.float32)
    assert x.shape == (B, C, H, W)
    xs = np.split(x, N_CORES, axis=0)
    return [
        {"x": xs[i], "w": w, "gamma": gamma, "beta": beta} for i in range(N_CORES)
    ]


def kernel(x, w, gamma, beta):
    in_maps = _make_in_maps(x, w, gamma, beta)
    res = _run(in_maps, trace=False)
    return np.concatenate([r["y"] for r in res.results], axis=0)


# ---- profiling helpers (used by test.py only) -------------------------

def _install_ntff_hook_shim():
    """bass_utils wants antenv.axon_hooks for NTFF tracing under axon; shim it."""
    import sys
    import types

    import antenv

    if "antenv.axon_hooks" in sys.modules:
        return
    mod = types.ModuleType("antenv.axon_hooks")
    mod._hook = None
    mod.set_axon_ntff_profile_hook = lambda h: setattr(mod, "_hook", h)
    mod.get_axon_ntff_profile_hook = lambda: mod._hook
    sys.modules["antenv.axon_hooks"] = mod
    antenv.axon_hooks = mod

    from trn_agent_boot.trn_boot import _ntff_profile_via_ctypes

    mod.set_axon_ntff_profile_hook(
        _ntff_profile_via_ctypes("/opt/axon/libaxon_pjrt.so")
    )


def kernel_traced(x, w, gamma, beta, tmpdir=None):
    """Run once with NTFF profiling; returns (y_full, exec_time_ns, trace_path)."""
    import concourse.bass_utils as bass_utils

    _install_ntff_hook_shim()
    bass_utils.upload_artifacts = lambda d: "local://disabled"
    in_maps = _make_in_maps(x, w, gamma, beta)
    res = _run(in_maps, trace=True, tmpdir=tmpdir)
    y = np.concatenate([r["y"] for r in res.results], axis=0)
    trace_path = (
        res.instructions_and_trace[1] if res.instructions_and_trace else None
    )
    return y, res.exec_time_ns, trace_path


# revision 25
# speedup vs baseline: 1.3113x; 1.3113x over previous
"""BinaryConvBNReLU Trainium2 kernel (8 NeuronCores, data-parallel over batch).

Reference computation (per nn.Module):
  bx = sign(x);  wc = clip(w, -1, 1);  alpha = mean(|wc|);  bw = sign(wc) * alpha
  out = conv2d(bx, bw, stride 1, pad 1) + x          (identity shortcut)
  out = batchnorm(out, batch stats over (B, H, W), gamma, beta, eps=1e-5)
  y = relu(out)

Strategy:
  - Batch sharded 4 images/core; weights replicated per core.
  - conv(sign x, sign w) on TensorE as 9 shifted fp8 DoubleRow matmuls per
    output-channel chunk (+-1 exact in fp8e4; contract dim 256 per matmul via
    [128, 2, N] paired operands; PSUM accumulates exact integers). The padded
    sign(x) image is stored flat (58-wide rows) so each tap's moving operand
    is one contiguous run; row-wrap garbage lands only in the 2 padding
    columns of each output row, skipped at PSUM eviction.
  - alpha (mean |clip(w)|) folded in at eviction: out = alpha*psum + x, with
    the per-channel BN sum fused in (accum_out); sum-of-squares on ScalarE.
  - Sync-BN: per-channel (sum, sumsq) AllReduced across the 8 cores (2 x 1KB),
    then y = relu(scale*out + bias) split across ScalarE/VectorE and streamed
    out. The whole per-core activation tensor lives in SBUF throughout.
  - Emission order is tuned so no strict-in-order engine FIFO blocks another
    engine's upcoming work (see inline comments).
"""

import numpy as np

B, C, H, W = 32, 256, 56, 56
K = 3
EPS = 1e-5
N_CORES = 8
B_LOC = B // N_CORES          # 4 images per core
P = 128                       # SBUF partitions
NCH = C // P                  # 2 channel chunks
HW = H * W                    # 3136
HP, WP = H + 2, W + 2         # 58x58 zero-padded sign(x) layout
ROWS = 8                      # output rows per PSUM tile
NRT = H // ROWS               # 7 row tiles per image
NT = ROWS * W                 # 448 pixels per PSUM tile (<=512 fp32 bank)
COUNT = B * HW                # BN reduction count (global batch)

_CACHE = {}


def _build_nc():
    import concourse.bacc as bacc
    import concourse.bass_isa as bass_isa
    import concourse.mybir as mybir
    import concourse.tile as tile
    from concourse.masks import make_identity
    from contextlib import ExitStack

    f32 = mybir.dt.float32
    bf16 = mybir.dt.bfloat16
    f8 = mybir.dt.float8e4
    Alu = mybir.AluOpType
    Act = mybir.ActivationFunctionType
    AxisX = mybir.AxisListType.X
    DR = mybir.MatmulPerfMode.DoubleRow

    # flat padded sign(x) layout: BASE leading zeros + 58*58 image (+ tail pad)
    # so every (kh, kw) tap window is one contiguous run (row-wrap garbage only
    # pollutes the 2 padding columns, which eviction skips). XLEN % 16 == 0
    # keeps the fp8 DoubleRow pair-step constraint satisfied.
    BASE = 16
    XLEN = 3392  # 16 + 58*58 + 12

    nc = bacc.Bacc(
        "TRN2", target_bir_lowering=False, debug=False, num_devices=N_CORES
    )
    x_d = nc.dram_tensor("x", [B_LOC, C, H, W], f32, kind="ExternalInput")
    w_d = nc.dram_tensor("w", [C, C, K, K], f32, kind="ExternalInput")
    g_d = nc.dram_tensor("gamma", [C], f32, kind="ExternalInput")
    be_d = nc.dram_tensor("beta", [C], f32, kind="ExternalInput")
    y_d = nc.dram_tensor("y", [B_LOC, C, H, W], f32, kind="ExternalOutput")

    with tile.TileContext(nc) as tc, ExitStack() as es:
        big = es.enter_context(tc.tile_pool(name="big", bufs=1))
        wpool = es.enter_context(tc.tile_pool(name="wpool", bufs=1))
        wst = es.enter_context(tc.tile_pool(name="wst", bufs=1))
        sgt = es.enter_context(tc.tile_pool(name="sgt", bufs=2))
        xpadp = es.enter_context(tc.tile_pool(name="xpadp", bufs=B_LOC))
        psum = es.enter_context(tc.tile_pool(name="psum", bufs=4, space="PSUM"))
        psum_sq = es.enter_context(tc.tile_pool(name="psum_sq", bufs=2, space="PSUM"))
        psum_t = es.enter_context(tc.tile_pool(name="psum_t", bufs=2, space="PSUM"))
        dram = es.enter_context(tc.tile_pool(name="dram", bufs=1, space="DRAM"))

        # Entire per-core activation tensor (x, then conv+x, then relu output)
        # stays resident in SBUF: [128, 4 img, 2 chunks, 3136 px] fp32.
        out_sb = big.tile([P, B_LOC, NCH, HW], f32, name="out_sb")
        # Transposed sign weights for fp8 DoubleRow: [ci_local, tap, ci_chunk, co].
        wT8 = wpool.tile([P, K * K, NCH, C], f8, name="wT8")
        identity = wpool.tile([P, P], bf16, name="identity")
        make_identity(nc, identity)

        gamma_sb = wpool.tile([P, NCH], f32, name="gamma_sb")
        nc.sync.dma_start(gamma_sb[:], g_d.ap().rearrange("(j p) -> p j", p=P))
        beta_sb = wpool.tile([P, NCH], f32, name="beta_sb")
        nc.sync.dma_start(beta_sb[:], be_d.ap().rearrange("(j p) -> p j", p=P))

        sum_stat = wpool.tile([P, NCH, B_LOC * NRT], f32, name="sum_stat")
        sq_stat = wpool.tile([P, NCH, B_LOC * NRT], f32, name="sq_stat")
        eps_sb = wpool.tile([P, 1], f32, name="eps_sb")
        nc.vector.memset(eps_sb[:], EPS)

        # ---- weight preprocessing (chunked, interleaved with x setup) --
        w_flat = w_d.ap().rearrange("o i kh kw -> o (i kh kw)")
        a_parts = wpool.tile([P, NCH], f32, name="a_parts")
        x_flat = x_d.ap().rearrange("b c h w -> b c (h w)")
        y_flat = y_d.ap().rearrange("b c h w -> b c (h w)")
        stats_loc = wpool.tile([P, NCH, 2], f32, name="stats_loc")

        w_sbs = []
        for j in range(NCH):
            w_sb = wst.tile([P, C * K * K], f32, tag="wsb", name=f"wsb{j}")
            w_sbs.append(w_sb)

        def w_dma(j):
            nc.sync.dma_start(w_sbs[j][:], w_flat[j * P : (j + 1) * P, :])

        def w_prep(j):
            w_sb = w_sbs[j]
            w_taps = w_sb.rearrange("p (c t) -> p t c", t=K * K)
            # sign(w) -> bf16, tap-major layout [co_local, tap, ci]; one ACT op
            # per tap so TensorE transposes can start after the first tap.
            sgn = sgt.tile([P, K * K, C], bf16, tag="sgn", name=f"sgn{j}")
            for t in range(K * K):
                nc.scalar.activation(sgn[:, t, :], w_taps[:, t, :], Act.Sign)
                # transpose each [co,ci] 128x128 block on TensorE -> [ci, co]
                for k in range(NCH):
                    pt = psum_t.tile([P, P], bf16, tag="pt", name=f"pt{j}_{t}_{k}")
                    nc.tensor.transpose(pt[:], sgn[:, t, k * P : (k + 1) * P], identity[:])
                    # PSUM->SBUF copy casts to fp8; on DVE so ScalarE stays
                    # free for the x sign passes
                    nc.vector.tensor_copy(wT8[:, t, k, j * P : (j + 1) * P], pt[:])

        def w_clip_reduce(j):
            w_sb = w_sbs[j]
            # clip(w, -1, 1) in place (sign unchanged; needed for alpha only)
            nc.vector.tensor_scalar(w_sb[:], w_sb[:], 1.0, -1.0, Alu.min, Alu.max)
            nc.vector.tensor_reduce(
                a_parts[:, j : j + 1],
                w_sb[:],
                axis=AxisX,
                op=Alu.add,
                apply_absolute_value=True,
            )

        xpads = [
            xpadp.tile([P, NCH, XLEN], f8, tag="xpad", name=f"xpad{b}")
            for b in range(B_LOC)
        ]

        def x_load(b, half=None):
            # half=0/1 loads the first/second 28 rows (kickstart pipelining)
            lo = 0 if half in (None, 0) else HW // 2
            hi = HW if half in (None, 1) else HW // 2
            for k in range(NCH):
                nc.sync.dma_start(
                    out_sb[:, b, k, lo:hi], x_flat[b, k * P : (k + 1) * P, lo:hi]
                )

        def x_sign(b, half=None):
            xpad = xpads[b]
            h0 = 0 if half in (None, 0) else H // 2
            h1 = H if half in (None, 1) else H // 2
            for k in range(NCH):
                pad_img = xpad[:, k, BASE : BASE + HP * WP].rearrange(
                    "p (r c) -> p r c", c=WP
                )
                nc.scalar.activation(
                    pad_img[:, h0 + 1 : h1 + 1, 1 : W + 1],
                    out_sb[:, b, k, h0 * W : h1 * W].rearrange(
                        "p (h w) -> p h w", w=W
                    ),
                    Act.Sign,
                )

        # alpha on DVE+gpsimd only (ACT stays free for sign passes)
        a_sum = wpool.tile([P, 1], f32, name="a_sum")
        a_all = wpool.tile([P, 1], f32, name="a_all")
        alpha = wpool.tile([P, 1], f32, name="alpha")

        def alpha_finalize():
            nc.vector.tensor_reduce(a_sum[:], a_parts[:], axis=AxisX, op=Alu.add)
            nc.gpsimd.partition_all_reduce(
                a_all[:], a_sum[:], channels=P, reduce_op=bass_isa.ReduceOp.add
            )
            nc.vector.tensor_scalar_mul(alpha[:], a_all[:], 1.0 / (C * C * K * K))

        def square_unit(j, b):
            # sum-of-squares pass (ScalarE) for one (chunk, image); emission
            # deferred for early units to keep ScalarE free for sign passes
            for rt in range(NRT):
                idx = b * NRT + rt
                sq = psum_sq.tile([P, NT], f32, tag="sq", name=f"sq{b}_{j}_{rt}")
                nc.scalar.activation(
                    sq[:], out_sb[:, b, j, rt * NT : (rt + 1) * NT], Act.Square,
                    accum_out=sq_stat[:, j, idx : idx + 1],
                )

        def conv_chunk_image(j, b, squares=True):
            xpad = xpads[b]
            for rt in range(NRT):
                # padded-width output tile [8 rows, 58 cols]; cols 0 and 57 are
                # row-wrap garbage and are skipped at eviction.
                ps = psum.tile([P, ROWS, WP], f32, tag="ps", name=f"ps{b}_{j}_{rt}")
                mm = 0
                for kh in range(K):
                    for kw in range(K):
                        s = BASE + (rt * ROWS + kh) * WP + (kw - 1)
                        nc.tensor.matmul(
                            ps[:],
                            wT8[:, kh * K + kw, :, j * P : (j + 1) * P],
                            xpad[:, :, s : s + ROWS * WP],
                            start=(mm == 0),
                            stop=(mm == K * K - 1),
                            perf_mode=DR,
                        )
                        mm += 1
                idx = b * NRT + rt
                sl = out_sb[:, b, j, rt * NT : (rt + 1) * NT].rearrange(
                    "p (r c) -> p r c", c=W
                )
                # out = alpha*conv + x (in place over x), accum -> per-tile sum
                nc.vector.scalar_tensor_tensor(
                    out=sl,
                    in0=ps[:, :, 1 : W + 1],
                    scalar=alpha[:],
                    in1=sl,
                    op0=Alu.mult,
                    op1=Alu.add,
                    accum_out=sum_stat[:, j, idx : idx + 1],
                )
            if squares:
                square_unit(j, b)

        def launch_allreduce(j):
            nc.vector.tensor_reduce(
                stats_loc[:, j, 0:1], sum_stat[:, j, :], axis=AxisX, op=Alu.add
            )
            nc.vector.tensor_reduce(
                stats_loc[:, j, 1:2], sq_stat[:, j, :], axis=AxisX, op=Alu.add
            )
            bnc_in = dram.tile([P, 2], f32, name=f"bncin{j}")
            bnc_out = dram.tile([P, 2], f32, name=f"bncout{j}", addr_space="Shared")
            nc.gpsimd.dma_start(bnc_in[:], stats_loc[:, j, :])
            nc.gpsimd.collective_compute(
                "AllReduce",
                Alu.add,
                replica_groups=[list(range(N_CORES))],
                ins=[bnc_in.opt()],
                outs=[bnc_out.opt()],
            )
            glob = wpool.tile([P, 2], f32, name=f"glob{j}")
            nc.gpsimd.dma_start(glob[:], bnc_out[:])
            return glob

        def normalize_store(j, glob):
            mean = wpool.tile([P, 1], f32, name=f"mean{j}")
            nc.scalar.mul(mean[:], glob[:, 0:1], 1.0 / COUNT)
            ex2 = wpool.tile([P, 1], f32, name=f"ex2{j}")
            nc.scalar.mul(ex2[:], glob[:, 1:2], 1.0 / COUNT)
            msq = wpool.tile([P, 1], f32, name=f"msq{j}")
            nc.vector.tensor_mul(msq[:], mean[:], mean[:])
            var = wpool.tile([P, 1], f32, name=f"var{j}")
            nc.vector.tensor_sub(var[:], ex2[:], msq[:])
            sd = wpool.tile([P, 1], f32, name=f"sd{j}")
            nc.scalar.activation(sd[:], var[:], Act.Sqrt, bias=eps_sb[:])
            rinv = wpool.tile([P, 1], f32, name=f"rinv{j}")
            nc.vector.reciprocal(rinv[:], sd[:])
            scl = wpool.tile([P, 1], f32, name=f"scl{j}")
            nc.vector.tensor_mul(scl[:], rinv[:], gamma_sb[:, j : j + 1])
            mscl = wpool.tile([P, 1], f32, name=f"mscl{j}")
            nc.vector.tensor_mul(mscl[:], mean[:], scl[:])
            bia = wpool.tile([P, 1], f32, name=f"bia{j}")
            nc.vector.tensor_sub(bia[:], beta_sb[:, j : j + 1], mscl[:])
            hh = HW // 2
            for b in range(B_LOC):
                for h in range(2):
                    sl = out_sb[:, b, j, h * hh : (h + 1) * hh]
                    if (2 * b + h) % 8 < 5:
                        # ScalarE: relu(scale*x + bias) in one op
                        nc.scalar.activation(
                            sl, sl, Act.Relu, bias=bia[:], scale=scl[:]
                        )
                    else:
                        # VectorE picks up the rest in parallel (2 ops)
                        nc.vector.tensor_scalar(
                            sl, sl, scl[:], bia[:], Alu.mult, Alu.add
                        )
                        nc.vector.tensor_scalar_max(sl, sl, 0.0)
                    nc.sync.dma_start(
                        y_flat[b, j * P : (j + 1) * P, h * hh : (h + 1) * hh], sl
                    )

        # Emission order tuned so no engine FIFO blocks another engine's
        # upcoming work (every engine is strict in-order):
        #  - DMA queue: w0, x(b0), w1, x(b1..b3)
        #  - ScalarE:  w-taps j0, signs b0, w-taps j1, signs b1..b3, squares...
        #  - TensorE:  transposes j0, conv(0,b0), transposes j1, conv...
        #  - VectorE:  w-copies j0, clips/alpha, evictions, w-copies j1, ...
        # Chunk 0 finishes six units in, so its AllReduce latency (incl. peer
        # launch skew) hides under the remaining chunk-1 conv, and chunk 0's
        # normalize+store hide under the chunk-1 AllReduce wait.
        for b in range(B_LOC):
            nc.gpsimd.memset(xpads[b][:], 0.0)
        w_dma(0)
        x_load(0, half=0)
        x_load(0, half=1)
        w_dma(1)
        for b in range(1, B_LOC):
            x_load(b, half=0)
            x_load(b, half=1)
        w_prep(0)
        x_sign(0, half=0)
        x_sign(0, half=1)
        w_clip_reduce(0)
        w_clip_reduce(1)
        alpha_finalize()
        conv_chunk_image(0, 0, squares=False)
        w_prep(1)
        x_sign(1, half=0)
        x_sign(1, half=1)
        conv_chunk_image(1, 0, squares=False)
        x_sign(2, half=0)
        x_sign(2, half=1)
        conv_chunk_image(0, 1, squares=False)
        x_sign(3, half=0)
        x_sign(3, half=1)
        square_unit(0, 0)
        square_unit(1, 0)
        square_unit(0, 1)
        conv_chunk_image(1, 1)
        conv_chunk_image(0, 2)
        conv_chunk_image(1, 2)
        conv_chunk_image(0, 3)
        glob0 = launch_allreduce(0)
        conv_chunk_image(1, 3)
        glob1 = launch_allreduce(1)
        normalize_store(0, glob0)
        normalize_store(1, glob1)

    nc.compile()
    return nc


def _get_nc():
    if "nc" not in _CACHE:
        _CACHE["nc"] = _build_nc()
    return _CACHE["nc"]


def _run(in_maps, trace=False, tmpdir=None):
    import concourse.bass_utils as bass_utils

    nc = _get_nc()
    return bass_utils.run_bass_kernel_spmd(
        nc, in_maps, core_ids=list(range(N_CORES)), trace=trace, tmpdir=tmpdir
    )


def _make_in_maps(x, w, gamma, beta):
    x = np.ascontiguousarray(np.asarray(x), dtype=np.float32)
    w = np.ascontiguousarray(np.asarray(w), dtype=np.float32)
    gamma = np.ascontiguousarray(np.asarray(gamma), dtype=np.float32)
    beta = np.ascontiguousarray(np.asarray(beta), dtype=np.float32)
    assert x.shape == (B, C, H, W)
    xs = np.split(x, N_CORES, axis=0)
    return [
        {"x": xs[i], "w": w, "gamma": gamma, "beta": beta} for i in range(N_CORES)
    ]


def kernel(x, w, gamma, beta):
    in_maps = _make_in_maps(x, w, gamma, beta)
    res = _run(in_maps, trace=False)
    return np.concatenate([r["y"] for r in res.results], axis=0)


# ---- profiling helpers (used by test.py only) -------------------------

def _install_ntff_hook_shim():
    """bass_utils wants antenv.axon_hooks for NTFF tracing under axon; shim it."""
    import sys
    import types

    import antenv

    if "antenv.axon_hooks" in sys.modules:
        return
    mod = types.ModuleType("antenv.axon_hooks")
    mod._hook = None
    mod.set_axon_ntff_profile_hook = lambda h: setattr(mod, "_hook", h)
    mod.get_axon_ntff_profile_hook = lambda: mod._hook
    sys.modules["antenv.axon_hooks"] = mod
    antenv.axon_hooks = mod

    from trn_agent_boot.trn_boot import _ntff_profile_via_ctypes

    mod.set_axon_ntff_profile_hook(
        _ntff_profile_via_ctypes("/opt/axon/libaxon_pjrt.so")
    )


def kernel_traced(x, w, gamma, beta, tmpdir=None):
    """Run once with NTFF profiling; returns (y_full, exec_time_ns, trace_path)."""
    import concourse.bass_utils as bass_utils

    _install_ntff_hook_shim()
    bass_utils.upload_artifacts = lambda d: "local://disabled"
    in_maps = _make_in_maps(x, w, gamma, beta)
    res = _run(in_maps, trace=True, tmpdir=tmpdir)
    y = np.concatenate([r["y"] for r in res.results], axis=0)
    trace_path = (
        res.instructions_and_trace[1] if res.instructions_and_trace else None
    )
    return y, res.exec_time_ns, trace_path


# revision 30
# speedup vs baseline: 1.3146x; 1.0025x over previous
"""BinaryConvBNReLU Trainium2 kernel (8 NeuronCores, data-parallel over batch).

Reference computation (per nn.Module):
  bx = sign(x);  wc = clip(w, -1, 1);  alpha = mean(|wc|);  bw = sign(wc) * alpha
  out = conv2d(bx, bw, stride 1, pad 1) + x          (identity shortcut)
  out = batchnorm(out, batch stats over (B, H, W), gamma, beta, eps=1e-5)
  y = relu(out)

Strategy:
  - Batch sharded 4 images/core; weights replicated per core.
  - conv(sign x, sign w) on TensorE as 9 shifted fp8 DoubleRow matmuls per
    output-channel chunk (+-1 exact in fp8e4; contract dim 256 per matmul via
    [128, 2, N] paired operands; PSUM accumulates exact integers). The padded
    sign(x) image is stored flat (58-wide rows) so each tap's moving operand
    is one contiguous run; row-wrap garbage lands only in the 2 padding
    columns of each output row, skipped at PSUM eviction.
  - alpha (mean |clip(w)|) folded in at eviction: out = alpha*psum + x, with
    the per-channel BN sum fused in (accum_out); sum-of-squares on ScalarE.
  - Sync-BN: per-channel (sum, sumsq) AllReduced across the 8 cores (2 x 1KB),
    then y = relu(scale*out + bias) split across ScalarE/VectorE and streamed
    out. The whole per-core activation tensor lives in SBUF throughout.
  - Emission order is tuned so no strict-in-order engine FIFO blocks another
    engine's upcoming work (see inline comments).
"""

import numpy as np

B, C, H, W = 32, 256, 56, 56
K = 3
EPS = 1e-5
N_CORES = 8
B_LOC = B // N_CORES          # 4 images per core
P = 128                       # SBUF partitions
NCH = C // P                  # 2 channel chunks
HW = H * W                    # 3136
HP, WP = H + 2, W + 2         # 58x58 zero-padded sign(x) layout
ROWS = 8                      # output rows per PSUM tile
NRT = H // ROWS               # 7 row tiles per image
NT = ROWS * W                 # 448 pixels per PSUM tile (<=512 fp32 bank)
COUNT = B * HW                # BN reduction count (global batch)

_CACHE = {}


def _build_nc():
    import concourse.bacc as bacc
    import concourse.bass_isa as bass_isa
    import concourse.mybir as mybir
    import concourse.tile as tile
    from concourse.masks import make_identity
    from contextlib import ExitStack

    f32 = mybir.dt.float32
    bf16 = mybir.dt.bfloat16
    f8 = mybir.dt.float8e4
    Alu = mybir.AluOpType
    Act = mybir.ActivationFunctionType
    AxisX = mybir.AxisListType.X
    DR = mybir.MatmulPerfMode.DoubleRow

    # flat padded sign(x) layout: BASE leading zeros + 58*58 image (+ tail pad)
    # so every (kh, kw) tap window is one contiguous run (row-wrap garbage only
    # pollutes the 2 padding columns, which eviction skips). XLEN % 16 == 0
    # keeps the fp8 DoubleRow pair-step constraint satisfied.
    BASE = 16
    XLEN = 3392  # 16 + 58*58 + 12

    nc = bacc.Bacc(
        "TRN2", target_bir_lowering=False, debug=False, num_devices=N_CORES
    )
    x_d = nc.dram_tensor("x", [B_LOC, C, H, W], f32, kind="ExternalInput")
    w_d = nc.dram_tensor("w", [C, C, K, K], f32, kind="ExternalInput")
    g_d = nc.dram_tensor("gamma", [C], f32, kind="ExternalInput")
    be_d = nc.dram_tensor("beta", [C], f32, kind="ExternalInput")
    y_d = nc.dram_tensor("y", [B_LOC, C, H, W], f32, kind="ExternalOutput")

    with tile.TileContext(nc) as tc, ExitStack() as es:
        big = es.enter_context(tc.tile_pool(name="big", bufs=1))
        wpool = es.enter_context(tc.tile_pool(name="wpool", bufs=1))
        wst = es.enter_context(tc.tile_pool(name="wst", bufs=1))
        sgt = es.enter_context(tc.tile_pool(name="sgt", bufs=2))
        xpadp = es.enter_context(tc.tile_pool(name="xpadp", bufs=B_LOC))
        psum = es.enter_context(tc.tile_pool(name="psum", bufs=5, space="PSUM"))
        psum_sq = es.enter_context(tc.tile_pool(name="psum_sq", bufs=1, space="PSUM"))
        psum_t = es.enter_context(tc.tile_pool(name="psum_t", bufs=2, space="PSUM"))
        dram = es.enter_context(tc.tile_pool(name="dram", bufs=1, space="DRAM"))

        # Entire per-core activation tensor (x, then conv+x, then relu output)
        # stays resident in SBUF: [128, 4 img, 2 chunks, 3136 px] fp32.
        out_sb = big.tile([P, B_LOC, NCH, HW], f32, name="out_sb")
        # Transposed sign weights for fp8 DoubleRow: [ci_local, tap, ci_chunk, co].
        wT8 = wpool.tile([P, K * K, NCH, C], f8, name="wT8")
        identity = wpool.tile([P, P], bf16, name="identity")
        make_identity(nc, identity)

        gamma_sb = wpool.tile([P, NCH], f32, name="gamma_sb")
        nc.sync.dma_start(gamma_sb[:], g_d.ap().rearrange("(j p) -> p j", p=P))
        beta_sb = wpool.tile([P, NCH], f32, name="beta_sb")
        nc.sync.dma_start(beta_sb[:], be_d.ap().rearrange("(j p) -> p j", p=P))

        sum_stat = wpool.tile([P, NCH, B_LOC * NRT], f32, name="sum_stat")
        sq_stat = wpool.tile([P, NCH, B_LOC * NRT], f32, name="sq_stat")
        eps_sb = wpool.tile([P, 1], f32, name="eps_sb")
        nc.vector.memset(eps_sb[:], EPS)

        # ---- weight preprocessing (chunked, interleaved with x setup) --
        w_flat = w_d.ap().rearrange("o i kh kw -> o (i kh kw)")
        a_parts = wpool.tile([P, NCH], f32, name="a_parts")
        x_flat = x_d.ap().rearrange("b c h w -> b c (h w)")
        y_flat = y_d.ap().rearrange("b c h w -> b c (h w)")
        stats_loc = wpool.tile([P, NCH, 2], f32, name="stats_loc")

        w_sbs = []
        for j in range(NCH):
            w_sb = wst.tile([P, C * K * K], f32, tag="wsb", name=f"wsb{j}")
            w_sbs.append(w_sb)

        def w_dma(j):
            # SWDGE queue: weight loads run concurrently with the x-image
            # loads on the HWDGE (sync) queue instead of delaying them
            nc.gpsimd.dma_start(w_sbs[j][:], w_flat[j * P : (j + 1) * P, :])

        def w_prep(j):
            w_sb = w_sbs[j]
            w_taps = w_sb.rearrange("p (c t) -> p t c", t=K * K)
            # sign(w) -> bf16, tap-major layout [co_local, tap, ci]; one ACT op
            # per tap so TensorE transposes can start after the first tap.
            sgn = sgt.tile([P, K * K, C], bf16, tag="sgn", name=f"sgn{j}")
            for t in range(K * K):
                nc.scalar.activation(sgn[:, t, :], w_taps[:, t, :], Act.Sign)
                # transpose each [co,ci] 128x128 block on TensorE -> [ci, co]
                for k in range(NCH):
                    pt = psum_t.tile([P, P], bf16, tag="pt", name=f"pt{j}_{t}_{k}")
                    nc.tensor.transpose(pt[:], sgn[:, t, k * P : (k + 1) * P], identity[:])
                    # PSUM->SBUF copy casts to fp8; on DVE so ScalarE stays
                    # free for the x sign passes
                    nc.vector.tensor_copy(wT8[:, t, k, j * P : (j + 1) * P], pt[:])

        def w_clip_reduce(j):
            w_sb = w_sbs[j]
            # clip(w, -1, 1) in place (sign unchanged; needed for alpha only)
            nc.vector.tensor_scalar(w_sb[:], w_sb[:], 1.0, -1.0, Alu.min, Alu.max)
            nc.vector.tensor_reduce(
                a_parts[:, j : j + 1],
                w_sb[:],
                axis=AxisX,
                op=Alu.add,
                apply_absolute_value=True,
            )

        xpads = [
            xpadp.tile([P, NCH, XLEN], f8, tag="xpad", name=f"xpad{b}")
            for b in range(B_LOC)
        ]

        def x_load(b, half=None):
            # half=0/1 loads the first/second 28 rows (kickstart pipelining)
            lo = 0 if half in (None, 0) else HW // 2
            hi = HW if half in (None, 1) else HW // 2
            for k in range(NCH):
                nc.sync.dma_start(
                    out_sb[:, b, k, lo:hi], x_flat[b, k * P : (k + 1) * P, lo:hi]
                )

        def x_sign(b, half=None):
            xpad = xpads[b]
            h0 = 0 if half in (None, 0) else H // 2
            h1 = H if half in (None, 1) else H // 2
            for k in range(NCH):
                pad_img = xpad[:, k, BASE : BASE + HP * WP].rearrange(
                    "p (r c) -> p r c", c=WP
                )
                nc.scalar.activation(
                    pad_img[:, h0 + 1 : h1 + 1, 1 : W + 1],
                    out_sb[:, b, k, h0 * W : h1 * W].rearrange(
                        "p (h w) -> p h w", w=W
                    ),
                    Act.Sign,
                )

        # alpha on DVE+gpsimd only (ACT stays free for sign passes)
        a_sum = wpool.tile([P, 1], f32, name="a_sum")
        a_all = wpool.tile([P, 1], f32, name="a_all")
        alpha = wpool.tile([P, 1], f32, name="alpha")

        def alpha_finalize():
            nc.vector.tensor_reduce(a_sum[:], a_parts[:], axis=AxisX, op=Alu.add)
            nc.gpsimd.partition_all_reduce(
                a_all[:], a_sum[:], channels=P, reduce_op=bass_isa.ReduceOp.add
            )
            nc.vector.tensor_scalar_mul(alpha[:], a_all[:], 1.0 / (C * C * K * K))

        def square_unit(j, b):
            # sum-of-squares pass (ScalarE) for one (chunk, image); emission
            # deferred for early units to keep ScalarE free for sign passes
            for rt in range(NRT):
                idx = b * NRT + rt
                sq = psum_sq.tile([P, NT], f32, tag="sq", name=f"sq{b}_{j}_{rt}")
                nc.scalar.activation(
                    sq[:], out_sb[:, b, j, rt * NT : (rt + 1) * NT], Act.Square,
                    accum_out=sq_stat[:, j, idx : idx + 1],
                )

        def conv_chunk_image(j, b, squares=True):
            xpad = xpads[b]
            for rt in range(NRT):
                # padded-width output tile [8 rows, 58 cols]; cols 0 and 57 are
                # row-wrap garbage and are skipped at eviction.
                ps = psum.tile([P, ROWS, WP], f32, tag="ps", name=f"ps{b}_{j}_{rt}")
                mm = 0
                for kh in range(K):
                    for kw in range(K):
                        s = BASE + (rt * ROWS + kh) * WP + (kw - 1)
                        nc.tensor.matmul(
                            ps[:],
                            wT8[:, kh * K + kw, :, j * P : (j + 1) * P],
                            xpad[:, :, s : s + ROWS * WP],
                            start=(mm == 0),
                            stop=(mm == K * K - 1),
                            perf_mode=DR,
                        )
                        mm += 1
                idx = b * NRT + rt
                sl = out_sb[:, b, j, rt * NT : (rt + 1) * NT].rearrange(
                    "p (r c) -> p r c", c=W
                )
                # out = alpha*conv + x (in place over x), accum -> per-tile sum
                nc.vector.scalar_tensor_tensor(
                    out=sl,
                    in0=ps[:, :, 1 : W + 1],
                    scalar=alpha[:],
                    in1=sl,
                    op0=Alu.mult,
                    op1=Alu.add,
                    accum_out=sum_stat[:, j, idx : idx + 1],
                )
            if squares:
                square_unit(j, b)

        def launch_allreduce(j):
            nc.vector.tensor_reduce(
                stats_loc[:, j, 0:1], sum_stat[:, j, :], axis=AxisX, op=Alu.add
            )
            nc.vector.tensor_reduce(
                stats_loc[:, j, 1:2], sq_stat[:, j, :], axis=AxisX, op=Alu.add
            )
            bnc_in = dram.tile([P, 2], f32, name=f"bncin{j}")
            bnc_out = dram.tile([P, 2], f32, name=f"bncout{j}", addr_space="Shared")
            nc.gpsimd.dma_start(bnc_in[:], stats_loc[:, j, :])
            nc.gpsimd.collective_compute(
                "AllReduce",
                Alu.add,
                replica_groups=[list(range(N_CORES))],
                ins=[bnc_in.opt()],
                outs=[bnc_out.opt()],
            )
            glob = wpool.tile([P, 2], f32, name=f"glob{j}")
            nc.gpsimd.dma_start(glob[:], bnc_out[:])
            return glob

        def normalize_store(j, glob):
            mean = wpool.tile([P, 1], f32, name=f"mean{j}")
            nc.scalar.mul(mean[:], glob[:, 0:1], 1.0 / COUNT)
            ex2 = wpool.tile([P, 1], f32, name=f"ex2{j}")
            nc.scalar.mul(ex2[:], glob[:, 1:2], 1.0 / COUNT)
            msq = wpool.tile([P, 1], f32, name=f"msq{j}")
            nc.vector.tensor_mul(msq[:], mean[:], mean[:])
            var = wpool.tile([P, 1], f32, name=f"var{j}")
            nc.vector.tensor_sub(var[:], ex2[:], msq[:])
            sd = wpool.tile([P, 1], f32, name=f"sd{j}")
            nc.scalar.activation(sd[:], var[:], Act.Sqrt, bias=eps_sb[:])
            rinv = wpool.tile([P, 1], f32, name=f"rinv{j}")
            nc.vector.reciprocal(rinv[:], sd[:])
            scl = wpool.tile([P, 1], f32, name=f"scl{j}")
            nc.vector.tensor_mul(scl[:], rinv[:], gamma_sb[:, j : j + 1])
            mscl = wpool.tile([P, 1], f32, name=f"mscl{j}")
            nc.vector.tensor_mul(mscl[:], mean[:], scl[:])
            bia = wpool.tile([P, 1], f32, name=f"bia{j}")
            nc.vector.tensor_sub(bia[:], beta_sb[:, j : j + 1], mscl[:])
            hh = HW // 2
            for b in range(B_LOC):
                for h in range(2):
                    sl = out_sb[:, b, j, h * hh : (h + 1) * hh]
                    if (2 * b + h) % 8 < 5:
                        # ScalarE: relu(scale*x + bias) in one op
                        nc.scalar.activation(
                            sl, sl, Act.Relu, bias=bia[:], scale=scl[:]
                        )
                    else:
                        # VectorE picks up the rest in parallel (2 ops)
                        nc.vector.tensor_scalar(
                            sl, sl, scl[:], bia[:], Alu.mult, Alu.add
                        )
                        nc.vector.tensor_scalar_max(sl, sl, 0.0)
                    nc.sync.dma_start(
                        y_flat[b, j * P : (j + 1) * P, h * hh : (h + 1) * hh], sl
                    )

        # Emission order tuned so no engine FIFO blocks another engine's
        # upcoming work (every engine is strict in-order):
        #  - DMA queue: w0, x(b0), w1, x(b1..b3)
        #  - ScalarE:  w-taps j0, signs b0, w-taps j1, signs b1..b3, squares...
        #  - TensorE:  transposes j0, conv(0,b0), transposes j1, conv...
        #  - VectorE:  w-copies j0, clips/alpha, evictions, w-copies j1, ...
        # Chunk 0 finishes six units in, so its AllReduce latency (incl. peer
        # launch skew) hides under the remaining chunk-1 conv, and chunk 0's
        # normalize+store hide under the chunk-1 AllReduce wait.
        w_dma(0)
        w_dma(1)
        for b in range(B_LOC):
            nc.gpsimd.memset(xpads[b][:], 0.0)
        x_load(0, half=0)
        x_load(0, half=1)
        for b in range(1, B_LOC):
            x_load(b, half=0)
            x_load(b, half=1)
        w_prep(0)
        x_sign(0, half=0)
        x_sign(0, half=1)
        w_clip_reduce(0)
        w_clip_reduce(1)
        alpha_finalize()
        conv_chunk_image(0, 0, squares=False)
        w_prep(1)
        x_sign(1, half=0)
        x_sign(1, half=1)
        conv_chunk_image(1, 0, squares=False)
        x_sign(2, half=0)
        x_sign(2, half=1)
        conv_chunk_image(0, 1, squares=False)
        x_sign(3, half=0)
        x_sign(3, half=1)
        square_unit(0, 0)
        square_unit(1, 0)
        square_unit(0, 1)
        conv_chunk_image(1, 1)
        conv_chunk_image(0, 2)
        conv_chunk_image(1, 2)
        conv_chunk_image(0, 3)
        glob0 = launch_allreduce(0)
        conv_chunk_image(1, 3)
        glob1 = launch_allreduce(1)
        normalize_store(0, glob0)
        normalize_store(1, glob1)

    nc.compile()
    return nc


def _get_nc():
    if "nc" not in _CACHE:
        _CACHE["nc"] = _build_nc()
    return _CACHE["nc"]


def _run(in_maps, trace=False, tmpdir=None):
    import concourse.bass_utils as bass_utils

    nc = _get_nc()
    return bass_utils.run_bass_kernel_spmd(
        nc, in_maps, core_ids=list(range(N_CORES)), trace=trace, tmpdir=tmpdir
    )


def _make_in_maps(x, w, gamma, beta):
    x = np.ascontiguousarray(np.asarray(x), dtype=np.float32)
    w = np.ascontiguousarray(np.asarray(w), dtype=np.float32)
    gamma = np.ascontiguousarray(np.asarray(gamma), dtype=np.float32)
    beta = np.ascontiguousarray(np.asarray(beta), dtype=np.float32)
    assert x.shape == (B, C, H, W)
    xs = np.split(x, N_CORES, axis=0)
    return [
        {"x": xs[i], "w": w, "gamma": gamma, "beta": beta} for i in range(N_CORES)
    ]


def kernel(x, w, gamma, beta):
    in_maps = _make_in_maps(x, w, gamma, beta)
    res = _run(in_maps, trace=False)
    return np.concatenate([r["y"] for r in res.results], axis=0)


# ---- profiling helpers (used by test.py only) -------------------------

def _install_ntff_hook_shim():
    """bass_utils wants antenv.axon_hooks for NTFF tracing under axon; shim it."""
    import sys
    import types

    import antenv

    if "antenv.axon_hooks" in sys.modules:
        return
    mod = types.ModuleType("antenv.axon_hooks")
    mod._hook = None
    mod.set_axon_ntff_profile_hook = lambda h: setattr(mod, "_hook", h)
    mod.get_axon_ntff_profile_hook = lambda: mod._hook
    sys.modules["antenv.axon_hooks"] = mod
    antenv.axon_hooks = mod

    from trn_agent_boot.trn_boot import _ntff_profile_via_ctypes

    mod.set_axon_ntff_profile_hook(
        _ntff_profile_via_ctypes("/opt/axon/libaxon_pjrt.so")
    )


def kernel_traced(x, w, gamma, beta, tmpdir=None):
    """Run once with NTFF profiling; returns (y_full, exec_time_ns, trace_path)."""
    import concourse.bass_utils as bass_utils

    _install_ntff_hook_shim()
    bass_utils.upload_artifacts = lambda d: "local://disabled"
    in_maps = _make_in_maps(x, w, gamma, beta)
    res = _run(in_maps, trace=True, tmpdir=tmpdir)
    y = np.concatenate([r["y"] for r in res.results], axis=0)
    trace_path = (
        res.instructions_and_trace[1] if res.instructions_and_trace else None
    )
    return y, res.exec_time_ns, trace_path
